# revision 1
# baseline (speedup 1.0000x reference)
"""DCE loss kernel for Trainium2 (8 NeuronCores, SPMD via bass).

loss[b] = cnt[c_b] * log(sum_p exp(-dist[b,p])) + sum_{p in class(b)} dist[b,p]

Device computes, per core (protos strided-sharded after a class sort):
  - u[b, p] = |x_b - p_p|^2 via two accumulating matmuls (fp32r)
  - dist = Sqrt(u + x2[b])   (ACT, bias per-partition, reads PSUM)
  - per-class column-range sums of dist (DVE reduce_sum)  -> msum
  - exp(-dist) in-place (ACT) + row sums (DVE)            -> onec
Host does the sort/pad/shard prep, final log/gather/combine, and unsort.
"""

import sys

import numpy as np

sys.path.insert(0, "/opt/trn_rl_repo")

import concourse.bass as bass  # noqa: E402
import concourse.bacc as bacc  # noqa: E402
import concourse.mybir as mybir  # noqa: E402
import concourse.tile as tile  # noqa: E402
from concourse.bass_utils import run_bass_kernel_spmd  # noqa: E402
from concourse.tile_rust import add_dep_helper  # noqa: E402

F32 = mybir.dt.float32
F16 = mybir.dt.float16
BF16 = mybir.dt.bfloat16
F32R = mybir.dt.float32r
ACT = mybir.ActivationFunctionType

NCORES = 8
PADV = 100.0  # pad-prototype first coordinate (rest zeros)

# knobs (test.py pokes these)
TRACE = False
SIM = False
LAST_EXEC_NS = None
LAST_RESULTS = None
LAST_TRACE = None

_BUILD_CACHE = {}

# variant knobs (analyze.py / test.py sweep these)
DIST16 = True  # fp16 dist + bf16 exp output (False: fp32 everywhere)
NACC = 0  # ACT accumulator measures ~+2.4us/instr on HW - do not use
EGRP = 6  # bts per grouped exp instruction
NSPLIT = 6  # onec sub-reduces per bt: keeps each DVE op under the ~266ns
# threshold below which the post-op pipe DRAIN vanishes


def _build_program(B, W, ops, grp, reps=0):
    """Build the SPMD bass program. ops: tuple of (bt, sc, ec) masked-sum ops.
    reps>0 wraps the compute body in a For_i loop (benchmark variants)."""
    NBT = B // 128
    S = len(ops)
    nc = bacc.Bacc("TRN2", target_bir_lowering=False, debug=False)

    # matmul operands ride in one packed tensor: [pT | p2 | ones | xT].
    # Loaded as a small leading DMA (pT/p2/ones) + 4 xT column chunks so
    # the first matmuls can start before the whole 2MB xT lands.
    TW = B + 2 * W + 128
    pk_d = nc.dram_tensor("pack", [128, TW], F32R, kind="ExternalInput").ap()
    x2_d = nc.dram_tensor("x2c", [128, NBT], F32, kind="ExternalInput").ap()
    onec_d = nc.dram_tensor("onec", [128, NBT], F32, kind="ExternalOutput").ap()
    msum_d = nc.dram_tensor("msum", [128, max(S, 1)], F32, kind="ExternalOutput").ap()

    # bank-aligned matmul chunks (each within one 2KB PSUM bank)
    chunks = [(c, min(c + 512, W)) for c in range(0, W, 512)]

    ops_by_bt = {}
    for slot, (bt, sc, ec) in enumerate(ops):
        ops_by_bt.setdefault(bt, []).append((slot, sc, ec))

    from contextlib import ExitStack

    with tile.TileContext(nc) as tc, ExitStack() as ctx:
        const_p = ctx.enter_context(tc.tile_pool(name="const", bufs=1))
        psum_p = ctx.enter_context(tc.tile_pool(name="psum", bufs=2, space="PSUM"))
        dist_p = ctx.enter_context(tc.tile_pool(name="dist", bufs=1))
        out_p = ctx.enter_context(tc.tile_pool(name="outs", bufs=1))

        pk_sb = const_p.tile([128, TW], F32R, tag="pack")
        x2_sb = const_p.tile([128, NBT], F32, tag="x2")
        head = 2 * W + 128
        # Load order tuned so bt0's first sqrt can start early: x2 (bias),
        # ones + xT(bt0), pT, p2, then the remaining xT chunks.
        nc.sync.dma_start(x2_sb[:], x2_d[:])
        nc.sync.dma_start(  # ones + xT bt0
            pk_sb[:, 2 * W : head + 128], pk_d[:, 2 * W : head + 128]
        )
        nc.sync.dma_start(pk_sb[:, 0:W], pk_d[:, 0:W])  # pT
        nc.sync.dma_start(pk_sb[:, W : 2 * W], pk_d[:, W : 2 * W])  # p2
        xq = (B - 128) // 4
        for q in range(4):  # rest of xT
            lo = head + 128 + q * xq
            hi = head + 128 + (q + 1) * xq if q < 3 else TW
            nc.sync.dma_start(pk_sb[:, lo:hi], pk_d[:, lo:hi])
        pT_sb = pk_sb[:, 0:W]
        p2_sb = pk_sb[0:1, W : 2 * W]
        ones_sb = pk_sb[0:1, 2 * W : 2 * W + 128]
        xT_sb = pk_sb[:, head : head + B]

        onec_sb = out_p.tile([128, NBT], F32, tag="onec")
        onec_scr_sb = out_p.tile([128, NBT, NSPLIT], F32, tag="onecscr")
        msum_sb = out_p.tile([128, max(S, 1)], F32, tag="msum")

        # dist for ALL bts stays resident (fp16: 83.5KB/partition) so the
        # sqrt and exp phases each run under a single ACT table load.
        dist_sb = dist_p.tile([128, NBT * W], F16 if DIST16 else F32, tag="dist")

        from contextlib import nullcontext

        loop_cm = tc.For_i(0, reps, 1) if reps else nullcontext()
        act_chain = []
        with loop_cm:
            body(nc, tc, NBT, grp, W, chunks, ops_by_bt, act_chain,
                 xT_sb, pT_sb, p2_sb, ones_sb, x2_sb, dist_sb, onec_sb,
                 msum_sb, psum_p, onec_scr_sb)

        # pin the ACT instruction order so sqrt/exp phases don't interleave
        # (a sqrt<->exp table switch costs ~1.3us each)
        for a, b in zip(act_chain, act_chain[1:]):
            add_dep_helper(b.ins, a.ins, sync=False, reason="act phase order")

        nc.sync.dma_start(onec_d[:], onec_sb[:])
        nc.sync.dma_start(msum_d[:], msum_sb[:])

    nc.compile()
    return nc


def body(nc, tc, NBT, grp, W, chunks, ops_by_bt, act_chain, xT_sb, pT_sb,
         p2_sb, ones_sb, x2_sb, dist_sb, onec_sb, msum_sb, psum_p,
         onec_scr_sb):
    ACT = mybir.ActivationFunctionType
    # ---- Phase A: matmul -> sqrt (dist) per bt; msum reduces trail on DVE
    for bt in range(NBT):
        u = psum_p.tile([128, W], F32, name="u", tag="u")
        for c0, c1 in chunks:
            nc.tensor.matmul(
                u[:, c0:c1],
                lhsT=xT_sb[:, bt * 128 : (bt + 1) * 128],
                rhs=pT_sb[:, c0:c1],
                start=True,
                stop=False,
            )
        for c0, c1 in chunks:
            nc.tensor.matmul(
                u[:, c0:c1], lhsT=ones_sb, rhs=p2_sb[:, c0:c1],
                start=False, stop=True,
            )
        dsl = dist_sb[:, bt * W : (bt + 1) * W]
        i_sqrt = nc.scalar.activation(
            dsl, u[:, 0:W], ACT.Sqrt, bias=x2_sb[:, bt : bt + 1], scale=1.0
        )
        act_chain.append(i_sqrt)
        for slot, sc, ec in ops_by_bt.get(bt, []):
            nc.vector.reduce_sum(
                msum_sb[:, slot : slot + 1],
                dist_sb[:, bt * W + sc : bt * W + ec],
                axis=mybir.AxisListType.X,
            )

    # ---- Phase B: exp. Grouped mega-instructions amortize ACT overhead;
    # their row sums run on DVE (1x reduce). The trailing NACC bts are
    # per-bt exp instructions whose row sum rides the ACT accumulator, so
    # the DVE finishes its reduces inside the ACT phase-B span (no tail).
    def eo(ap):
        # in-place exp; with fp16 dist, write bf16 over the same bytes
        # (bf16 avoids fp16 subnormal underflow at exp(-25))
        return ap.bitcast(BF16) if DIST16 else ap

    nacc = min(NACC, NBT)
    ngrp = NBT - nacc
    # per-bt partial sums land in onec_scr_sb[:, bt, :]; one segmented
    # reduce at the end folds them into onec (innermost X of the 3-D view)
    pos = 0
    while pos < ngrp:
        g = min(EGRP, ngrp - pos)
        dsl = dist_sb[:, pos * W : (pos + g) * W]
        i_exp = nc.scalar.activation(eo(dsl), dsl, ACT.Exp, scale=-1.0)
        act_chain.append(i_exp)
        for j in range(pos, pos + g):
            bounds = [round(W * i / NSPLIT) for i in range(NSPLIT + 1)]
            for s in range(NSPLIT):
                lo, hi = bounds[s], bounds[s + 1]
                nc.vector.reduce_sum(
                    onec_scr_sb[:, j, s : s + 1],
                    eo(dist_sb[:, j * W + lo : j * W + hi]),
                    axis=mybir.AxisListType.X,
                )
        pos += g
    for bt in range(ngrp, NBT):
        dsl = dist_sb[:, bt * W : (bt + 1) * W]
        i_exp = nc.scalar.activation(
            eo(dsl), dsl, ACT.Exp, scale=-1.0,
            accum_out=onec_sb[:, bt : bt + 1],
        )
        act_chain.append(i_exp)
    if ngrp:
        nc.vector.reduce_sum(
            onec_sb[:, 0:ngrp], onec_scr_sb[:, 0:ngrp, :],
            axis=mybir.AxisListType.X,
        )


class _Prep:
    """Host-side preparation: program + per-core inputs + result assembly."""

    def __init__(self, nc, in_maps, assemble):
        self.nc = nc
        self.in_maps = in_maps
        self.assemble = assemble


def prepare(feature, label, proto_features, proto_labels, reps=0):
    x = np.asarray(feature, dtype=np.float32)
    lab = np.asarray(label).astype(np.int64)
    pf = np.asarray(proto_features, dtype=np.float32)
    plab = np.asarray(proto_labels).astype(np.int64)

    B, D = x.shape
    P = pf.shape[0]
    assert D == 128 and B % 128 == 0
    NBT = B // 128

    # ---------------- host prep: sort protos by class, pad to mult of 8 ----
    order_p = np.argsort(plab, kind="stable")
    plab_s = plab[order_p]
    classes = np.unique(plab_s)
    nclass_max = int(plab_s.max()) + 1 if P else 1

    cnt = np.bincount(plab, minlength=nclass_max).astype(np.int64)

    blocks = []
    cls_grange = {}  # class -> (gstart, gend_padded)
    npad = np.zeros(nclass_max, dtype=np.int64)
    pos = 0
    pad_row = np.zeros((1, D), dtype=np.float32)
    pad_row[0, 0] = PADV
    for c in classes:
        lo = np.searchsorted(plab_s, c, "left")
        hi = np.searchsorted(plab_s, c, "right")
        idx = order_p[lo:hi]
        n = len(idx)
        npc = ((n + 7) // 8) * 8
        blocks.append(pf[idx])
        if npc > n:
            blocks.append(np.repeat(pad_row, npc - n, axis=0))
        npad[c] = npc - n
        cls_grange[int(c)] = (pos, pos + npc)
        pos += npc
    # global pad so W = pos/8 is even (fp32r matmul needs an even moving dim)
    gpad = (-pos) % (2 * NCORES)
    if gpad:
        blocks.append(np.repeat(pad_row, gpad, axis=0))
        pos += gpad
    padded = np.concatenate(blocks, axis=0)  # [pos, D], pos % 16 == 0
    W = pos // NCORES
    assert pos % NCORES == 0 and W % 2 == 0

    # strided shard: core k holds sorted-padded protos k::8
    pT_cores = []
    p2_cores = []
    for k in range(NCORES):
        sh = padded[k::NCORES]  # [W, D]
        pT_cores.append(np.ascontiguousarray(-2.0 * sh.T, dtype=np.float32))
        p2_cores.append((sh * sh).sum(1, dtype=np.float32))

    # ---------------- host prep: sort features by class -------------------
    order_b = np.argsort(lab, kind="stable")
    xs = x[order_b]
    lab_s = lab[order_b]
    xT = np.ascontiguousarray(xs.T, dtype=np.float32)  # [128, B]
    x2 = (xs * xs).sum(1, dtype=np.float32)  # [B]
    x2c = np.ascontiguousarray(x2.reshape(NBT, 128).T, dtype=np.float32)

    # ---------------- masked-op schedule (uniform across cores) -----------
    ops = []
    op_meta = []  # (bt, class, row_lo, row_hi)  rows local to bt
    for c in classes:
        g0, g1 = cls_grange[int(c)]
        sc, ec = g0 // NCORES, g1 // NCORES
        bs = int(np.searchsorted(lab_s, c, "left"))
        be = int(np.searchsorted(lab_s, c, "right"))
        if bs == be:
            continue
        for bt in range(bs // 128, (be + 127) // 128):
            ops.append((bt, sc, ec))
            rlo = max(bs, bt * 128) - bt * 128
            rhi = min(be, (bt + 1) * 128) - bt * 128
            op_meta.append((bt, int(c), rlo, rhi))
    ops = tuple(ops)
    S = len(ops)

    grp = NBT  # full phase split: one sqrt phase, one exp phase

    key = (B, W, ops, grp, reps, DIST16, NACC, EGRP, NSPLIT)
    if key not in _BUILD_CACHE:
        _BUILD_CACHE[key] = _build_program(B, W, ops, grp, reps)
    nc = _BUILD_CACHE[key]

    # packed matmul-operand tensor: [pT | p2(row0) | ones(row0) | xT]
    TW = B + 2 * W + 128
    in_maps = []
    for k in range(NCORES):
        pack = np.zeros((128, TW), dtype=np.float32)
        pack[:, 0:W] = pT_cores[k]
        pack[0, W : 2 * W] = p2_cores[k]
        pack[0, 2 * W : 2 * W + 128] = 1.0
        pack[:, 2 * W + 128 : 2 * W + 128 + B] = xT
        in_maps.append({"pack": pack, "x2c": x2c})

    def assemble(results):
        onec = np.zeros((128, NBT), dtype=np.float64)
        msum = np.zeros((128, max(S, 1)), dtype=np.float64)
        for r in results:
            onec += r["onec"].astype(np.float64)
            msum += r["msum"].astype(np.float64)

        one = onec.T.reshape(B)  # one[b], b in sorted order
        log_one = np.log(one)

        masked = np.zeros(B, dtype=np.float64)
        for slot, (bt, c, rlo, rhi) in enumerate(op_meta):
            masked[bt * 128 + rlo : bt * 128 + rhi] += msum[rlo:rhi, slot]

        # subtract pad-proto contributions (pad = (PADV, 0, ..., 0))
        d_pad = np.sqrt(x2.astype(np.float64) + PADV * PADV - 2.0 * PADV * xs[:, 0])
        masked -= npad[lab_s] * d_pad

        loss_s = cnt[lab_s] * log_one + masked
        loss = np.empty(B, dtype=np.float64)
        loss[order_b] = loss_s
        return loss.astype(np.float32)

    return _Prep(nc, in_maps, assemble)


def kernel(feature, label, proto_features, proto_labels):
    global LAST_EXEC_NS, LAST_RESULTS
    p = prepare(feature, label, proto_features, proto_labels)
    nc, in_maps = p.nc, p.in_maps

    if SIM:
        from concourse.bass_interp import CoreSim

        results = []
        for k in range(NCORES):
            sim = CoreSim(nc, trace=False)
            for name, arr in in_maps[k].items():
                sim.tensor(name)[:] = arr
            sim.simulate(check_with_hw=False)
            results.append(
                {
                    "onec": sim.tensor("onec").copy(),
                    "msum": sim.tensor("msum").copy(),
                }
            )
        LAST_EXEC_NS = None
    else:
        res = run_bass_kernel_spmd(nc, in_maps, list(range(NCORES)), trace=TRACE)
        results = res.results
        LAST_EXEC_NS = res.exec_time_ns
        global LAST_TRACE
        LAST_TRACE = res.instructions_and_trace
    LAST_RESULTS = results
    return p.assemble(results)



# revision 38
# speedup vs baseline: 1.8181x; 1.8181x over previous
"""DCE loss kernel for Trainium2 (8 NeuronCores, SPMD via bass).

loss[b] = cnt[c_b] * log(sum_p exp(-dist[b,p])) + sum_{p in class(b)} dist[b,p]

Device computes, per core (protos strided-sharded after a class sort):
  - u[b, p] = |x_b - p_p|^2 via two accumulating matmuls (fp32r)
  - dist = Sqrt(u + x2[b])   (ACT, bias per-partition, reads PSUM)
  - per-class column-range sums of dist (DVE reduce_sum)  -> msum
  - e = exp(12 - dist) in-place fp16 (ACT; +12 keeps values fp16-normal)
  - row sums of e via a DVE tensor_tensor add tree (2x fp16 mode, every op
    under the ~266ns drain threshold) + one small 1x reduce -> onec
Host does the sort/pad/shard prep, final log/gather/combine, and unsort.
"""

import sys

import numpy as np

sys.path.insert(0, "/opt/trn_rl_repo")

import concourse.bass as bass  # noqa: E402
import concourse.bacc as bacc  # noqa: E402
import concourse.mybir as mybir  # noqa: E402
import concourse.tile as tile  # noqa: E402
from concourse.bass_utils import run_bass_kernel_spmd  # noqa: E402
from concourse.tile_rust import add_dep_helper  # noqa: E402

F32 = mybir.dt.float32
F16 = mybir.dt.float16
BF16 = mybir.dt.bfloat16
F32R = mybir.dt.float32r
ACT = mybir.ActivationFunctionType

NCORES = 8
PADV = 100.0  # pad-prototype first coordinate (rest zeros)
EBIAS = 12.0  # exp computes exp(EBIAS - d): keeps outputs in fp16 normal range

# knobs (test.py pokes these)
TRACE = False
SIM = False
LAST_EXEC_NS = None
LAST_RESULTS = None
LAST_TRACE = None

_BUILD_CACHE = {}

# exp instruction group sizes (bts per ACT exp instruction): sized so the
# DVE add-trees pipeline behind the ACT exps with minimal end-of-phase tail
# (see sched_opt.py for the makespan model these came from)
EGROUPS = (6, 6, 6, 4, 3, 2, 2, 1, 1, 1)
# groups whose first POOLBTS bts run their add tree on the (otherwise idle)
# Pool engine instead of the DVE
POOLGRPS = (1, 2, 3, 4, 5, 6)
POOLBTS = 1
# a pool bt's final DVE reduce is emitted DEFER groups later so the slow Pool
# tree never head-of-line-blocks the in-order DVE queue
DEFER = 3


def _build_program(B, W, ops, reps=0):
    """Build the SPMD bass program. ops: tuple of (bt, sc, ec) masked-sum ops.
    reps>0 wraps the compute body in a For_i loop (benchmark variants)."""
    NBT = B // 128
    S = len(ops)
    assert W % 8 == 0
    nc = bacc.Bacc("TRN2", target_bir_lowering=False, debug=False)

    # bank-aligned matmul chunks (each within one 2KB PSUM bank)
    chunks = [(c, min(c + 512, W)) for c in range(0, W, 512)]
    NCH = len(chunks)

    assert NCH <= 3  # p2 chunk rows sit at partitions 0/32/64 (matmul rule)
    pT_d = nc.dram_tensor("pT", [128, W], F32R, kind="ExternalInput").ap()
    # p2 packed one matmul-chunk per 32nd partition row: the DMA moves short
    # parallel lines instead of one slow W-long single-partition line, and
    # each chunk row sits at a legal matmul base partition (0/32/64)
    p2_d = nc.dram_tensor("p2", [65, 512], F32R, kind="ExternalInput").ap()
    x2_d = nc.dram_tensor("x2c", [128, NBT], F32, kind="ExternalInput").ap()
    # xT split: bt0 alone (tiny, unblocks the first matmul), rest of quarter
    # 0, then quarters 1-3 (tile deps are tile-granular)
    XQ = B // 4
    xq0_d = nc.dram_tensor("xq00", [128, 128], F32R, kind="ExternalInput").ap()
    xqr_d = nc.dram_tensor("xq0r", [128, XQ - 128], F32R, kind="ExternalInput").ap()
    xq_d = [
        nc.dram_tensor(f"xq{q}", [128, XQ], F32R, kind="ExternalInput").ap()
        for q in range(1, 4)
    ]
    ones_d = nc.dram_tensor("ones", [65, 128], F32R, kind="ExternalInput").ap()
    cstb_d = nc.dram_tensor("cstb", [128, 2], F32, kind="ExternalInput").ap()
    onec_d = nc.dram_tensor("onec", [128, NBT], F32, kind="ExternalOutput").ap()
    msum_d = nc.dram_tensor("msum", [128, max(S, 1)], F32, kind="ExternalOutput").ap()

    ops_by_bt = {}
    for slot, (bt, sc, ec) in enumerate(ops):
        ops_by_bt.setdefault(bt, []).append((slot, sc, ec))

    from contextlib import ExitStack

    with tile.TileContext(nc) as tc, ExitStack() as ctx:
        const_p = ctx.enter_context(tc.tile_pool(name="const", bufs=1))
        psum_p = ctx.enter_context(tc.tile_pool(name="psum", bufs=2, space="PSUM"))
        dist_p = ctx.enter_context(tc.tile_pool(name="dist", bufs=1))
        out_p = ctx.enter_context(tc.tile_pool(name="outs", bufs=1))
        scr_p = ctx.enter_context(tc.tile_pool(name="scr", bufs=2))
        pscr_p = ctx.enter_context(tc.tile_pool(name="pscr", bufs=4))

        pT_sb = const_p.tile([128, W], F32R, tag="pT")
        p2_sb = const_p.tile([65, 512], F32R, tag="p2")
        x2_sb = const_p.tile([128, NBT], F32, tag="x2")
        ones_sb = const_p.tile([65, 128], F32R, tag="ones")
        xq0_sb = const_p.tile([128, 128], F32R, tag="xq00")
        xqr_sb = const_p.tile([128, XQ - 128], F32R, tag="xq0r")
        xq_sb = [
            const_p.tile([128, XQ], F32R, tag=f"xq{q}", name=f"xq{q}")
            for q in range(1, 4)
        ]
        dum_sb = const_p.tile([1, 2], F32, tag="dum")
        cstb_sb = const_p.tile([128, 2], F32, tag="cstb")

        # constants first (they unblock the t=0 dummy activation), then input
        # DMAs spread across the SP and gpsimd queues (ACT queue stays clean
        # so the sqrt table load runs immediately)
        eb_sb = cstb_sb[:, 0:1]
        dum_in = cstb_sb[0:1, 0:2]
        nc.gpsimd.dma_start(cstb_sb[:], cstb_d[:])
        nc.gpsimd.dma_start(ones_sb[:], ones_d[:])
        nc.sync.dma_start(pT_sb[:], pT_d[:])
        nc.gpsimd.dma_start(xq0_sb[:], xq0_d[:])
        nc.sync.dma_start(p2_sb[:], p2_d[:])
        nc.sync.dma_start(x2_sb[:], x2_d[:])
        nc.gpsimd.dma_start(xqr_sb[:], xqr_d[:])
        nc.sync.dma_start(xq_sb[0][:], xq_d[0][:])
        nc.gpsimd.dma_start(xq_sb[1][:], xq_d[1][:])
        nc.sync.dma_start(xq_sb[2][:], xq_d[2][:])

        onec_sb = out_p.tile([128, NBT], F32, tag="onec")
        msum_sb = out_p.tile([128, max(S, 1)], F32, tag="msum")

        # dist for ALL bts stays resident (fp16) so the sqrt and exp phases
        # each run under a single ACT table load.
        dist_sb = dist_p.tile([128, NBT * W], F16, tag="dist")

        from contextlib import nullcontext

        loop_cm = tc.For_i(0, reps, 1) if reps else nullcontext()
        act_chain = []
        # prime the Sqrt table load at t=0 with a tiny dummy activation whose
        # only dep is the ones memset
        i_dum = nc.scalar.activation(dum_sb[:], dum_in[:], ACT.Sqrt)
        act_chain.append(i_dum)
        with loop_cm:
            body(nc, tc, NBT, W, chunks, ops_by_bt, act_chain,
                 (xq0_sb, xqr_sb) + tuple(xq_sb), pT_sb, p2_sb, ones_sb,
                 x2_sb, dist_sb, onec_sb, msum_sb, psum_p, scr_p, pscr_p,
                 eb_sb)

        # pin the ACT instruction order so sqrt/exp phases don't interleave
        # (a sqrt<->exp table switch costs ~1.3us each)
        for a, b in zip(act_chain, act_chain[1:]):
            add_dep_helper(b.ins, a.ins, sync=False, reason="act phase order")

        nc.sync.dma_start(onec_d[:], onec_sb[:])
        nc.sync.dma_start(msum_d[:], msum_sb[:])

    nc.compile()
    return nc


def body(nc, tc, NBT, W, chunks, ops_by_bt, act_chain, xq_sb, pT_sb,
         p2_sb, ones_sb, x2_sb, dist_sb, onec_sb, msum_sb, psum_p, scr_p,
         pscr_p, eb_sb):
    ACT = mybir.ActivationFunctionType
    BPQ = NBT // 4  # bts per xT quarter
    # xq_sb = (xq00 [bt0], xq0r [bts 1-7], xq1, xq2, xq3)
    def xslice(bt):
        if bt == 0:
            return xq_sb[0][:, 0:128]
        if bt < BPQ:
            return xq_sb[1][:, (bt - 1) * 128 : bt * 128]
        q = bt // BPQ
        lo = (bt % BPQ) * 128
        return xq_sb[q + 1][:, lo : lo + 128]

    # ---- Phase A: matmul -> sqrt (dist) per bt; msum reduces trail on DVE
    for bt in range(NBT):
        u = psum_p.tile([128, W], F32, name="u", tag="u")
        for c0, c1 in chunks:
            nc.tensor.matmul(
                u[:, c0:c1],
                lhsT=xslice(bt),
                rhs=pT_sb[:, c0:c1],
                start=True,
                stop=False,
            )
        for ci, (c0, c1) in enumerate(chunks):
            nc.tensor.matmul(
                u[:, c0:c1], lhsT=ones_sb[32 * ci : 32 * ci + 1, :],
                rhs=p2_sb[32 * ci : 32 * ci + 1, 0 : c1 - c0],
                start=False, stop=True,
            )
        dsl = dist_sb[:, bt * W : (bt + 1) * W]
        i_sqrt = nc.scalar.activation(
            dsl, u[:, 0:W], ACT.Sqrt, bias=x2_sb[:, bt : bt + 1], scale=1.0
        )
        act_chain.append(i_sqrt)
        for slot, sc, ec in ops_by_bt.get(bt, []):
            nc.vector.reduce_sum(
                msum_sb[:, slot : slot + 1],
                dist_sb[:, bt * W + sc : bt * W + ec],
                axis=mybir.AxisListType.X,
            )

    # ---- Phase B: e = exp(EBIAS - dist) in-place fp16; row sums via a DVE
    # tensor_tensor add tree (2x fp16) + one small 1x reduce per bt. Every
    # DVE op stays under the ~266ns pipe-drain threshold.
    W4 = W // 4
    W8 = W // 8
    pos = 0
    deferred = {}  # emit-group-index -> list of (bt, tD tile) pool reduces
    ngroups = 0
    for gi, g in enumerate(EGROUPS):
        g = min(g, NBT - pos)
        if g <= 0:
            break
        ngroups = gi + 1
        dsl = dist_sb[:, pos * W : (pos + g) * W]
        i_exp = nc.scalar.activation(
            dsl, dsl, ACT.Exp, scale=-1.0, bias=eb_sb[:, 0:1]
        )
        act_chain.append(i_exp)
        # deferred reduces of pool bts from DEFER groups ago come first: their
        # Pool trees finished long ago, so they never stall the DVE queue
        for (j, tD) in deferred.pop(gi, []):
            nc.vector.reduce_sum(
                onec_sb[:, j : j + 1], tD[:], axis=mybir.AxisListType.X
            )
        for j in range(pos, pos + g):
            on_pool = gi in POOLGRPS and (j - pos) < POOLBTS
            eng = nc.gpsimd if on_pool else nc.vector
            pool = pscr_p if on_pool else scr_p
            pfx = "p" if on_pool else "v"
            e = dist_sb[:, j * W : (j + 1) * W]
            tA = pool.tile([128, W4], F16, name="tA", tag=pfx + "tA")
            tB = pool.tile([128, W4], F16, name="tB", tag=pfx + "tB")
            tC = pool.tile([128, W4], F16, name="tC", tag=pfx + "tC")
            tD = pool.tile([128, W8], F16, name="tD", tag=pfx + "tD")
            with nc.allow_low_precision(reason="fp16 pairwise sums, +12 bias"):
                eng.tensor_add(tA[:], e[:, 0:W4], e[:, W4 : 2 * W4])
                eng.tensor_add(tB[:], e[:, 2 * W4 : 3 * W4], e[:, 3 * W4 : W])
                eng.tensor_add(tC[:], tA[:], tB[:])
                eng.tensor_add(tD[:], tC[:, 0:W8], tC[:, W8:W4])
            if on_pool:
                deferred.setdefault(gi + DEFER, []).append((j, tD))
            else:
                nc.vector.reduce_sum(
                    onec_sb[:, j : j + 1], tD[:], axis=mybir.AxisListType.X
                )
        pos += g
    # any remaining deferred reduces
    for gi in sorted(deferred):
        for (j, tD) in deferred[gi]:
            nc.vector.reduce_sum(
                onec_sb[:, j : j + 1], tD[:], axis=mybir.AxisListType.X
            )


class _Prep:
    """Host-side preparation: program + per-core inputs + result assembly."""

    def __init__(self, nc, in_maps, assemble):
        self.nc = nc
        self.in_maps = in_maps
        self.assemble = assemble


def prepare(feature, label, proto_features, proto_labels, reps=0):
    x = np.asarray(feature, dtype=np.float32)
    lab = np.asarray(label).astype(np.int64)
    pf = np.asarray(proto_features, dtype=np.float32)
    plab = np.asarray(proto_labels).astype(np.int64)

    B, D = x.shape
    P = pf.shape[0]
    assert D == 128 and B % 128 == 0
    NBT = B // 128

    # ---------------- host prep: sort protos by class, pad to mult of 8 ----
    order_p = np.argsort(plab, kind="stable")
    plab_s = plab[order_p]
    classes = np.unique(plab_s)
    nclass_max = int(plab_s.max()) + 1 if P else 1

    cnt = np.bincount(plab, minlength=nclass_max).astype(np.int64)

    blocks = []
    cls_grange = {}  # class -> (gstart, gend_padded)
    npad = np.zeros(nclass_max, dtype=np.int64)
    pos = 0
    pad_row = np.zeros((1, D), dtype=np.float32)
    pad_row[0, 0] = PADV
    for c in classes:
        lo = np.searchsorted(plab_s, c, "left")
        hi = np.searchsorted(plab_s, c, "right")
        idx = order_p[lo:hi]
        n = len(idx)
        npc = ((n + 7) // 8) * 8
        blocks.append(pf[idx])
        if npc > n:
            blocks.append(np.repeat(pad_row, npc - n, axis=0))
        npad[c] = npc - n
        cls_grange[int(c)] = (pos, pos + npc)
        pos += npc
    # global pad so W = pos/8 is a multiple of 8 (fp32r needs even moving dim;
    # the DVE add tree needs W % 8 == 0)
    gpad = (-pos) % (8 * NCORES)
    if gpad:
        blocks.append(np.repeat(pad_row, gpad, axis=0))
        pos += gpad
    padded = np.concatenate(blocks, axis=0)  # [pos, D]
    W = pos // NCORES
    assert pos % NCORES == 0 and W % 8 == 0

    # strided shard: core k holds sorted-padded protos k::8
    pT_cores = []
    p2_cores = []
    for k in range(NCORES):
        sh = padded[k::NCORES]  # [W, D]
        pT_cores.append(np.ascontiguousarray(-2.0 * sh.T, dtype=np.float32))
        p2_cores.append((sh * sh).sum(1, dtype=np.float32))

    # ---------------- host prep: sort features by class -------------------
    order_b = np.argsort(lab, kind="stable")
    xs = x[order_b]
    lab_s = lab[order_b]
    xT = np.ascontiguousarray(xs.T, dtype=np.float32)  # [128, B]
    x2 = (xs * xs).sum(1, dtype=np.float32)  # [B]
    x2c = np.ascontiguousarray(x2.reshape(NBT, 128).T, dtype=np.float32)

    # ---------------- masked-op schedule (uniform across cores) -----------
    ops = []
    op_meta = []  # (bt, class, row_lo, row_hi)  rows local to bt
    for c in classes:
        g0, g1 = cls_grange[int(c)]
        sc, ec = g0 // NCORES, g1 // NCORES
        bs = int(np.searchsorted(lab_s, c, "left"))
        be = int(np.searchsorted(lab_s, c, "right"))
        if bs == be:
            continue
        for bt in range(bs // 128, (be + 127) // 128):
            ops.append((bt, sc, ec))
            rlo = max(bs, bt * 128) - bt * 128
            rhi = min(be, (bt + 1) * 128) - bt * 128
            op_meta.append((bt, int(c), rlo, rhi))
    ops = tuple(ops)
    S = len(ops)

    key = (B, W, ops, reps, EGROUPS)
    if key not in _BUILD_CACHE:
        _BUILD_CACHE[key] = _build_program(B, W, ops, reps)
    nc = _BUILD_CACHE[key]

    XQ = B // 4
    NCH = (W + 511) // 512
    in_maps = []
    for k in range(NCORES):
        p2pack = np.zeros((65, 512), dtype=np.float32)
        for ci in range(NCH):
            c0, c1 = ci * 512, min(ci * 512 + 512, W)
            p2pack[32 * ci, 0 : c1 - c0] = p2_cores[k][c0:c1]
        cstb = np.zeros((128, 2), dtype=np.float32)
        cstb[:, 0] = EBIAS
        m = {
            "pT": pT_cores[k],
            "p2": p2pack,
            "x2c": x2c,
            "xq00": np.ascontiguousarray(xT[:, 0:128]),
            "xq0r": np.ascontiguousarray(xT[:, 128:XQ]),
            "ones": np.ones((65, 128), dtype=np.float32),
            "cstb": cstb,
        }
        for q in range(1, 4):
            m[f"xq{q}"] = np.ascontiguousarray(xT[:, q * XQ : (q + 1) * XQ])
        in_maps.append(m)

    def assemble(results):
        onec = np.zeros((128, NBT), dtype=np.float64)
        msum = np.zeros((128, max(S, 1)), dtype=np.float64)
        for r in results:
            onec += r["onec"].astype(np.float64)
            msum += r["msum"].astype(np.float64)

        one = onec.T.reshape(B)  # one[b] * e^EBIAS, b in sorted order
        log_one = np.log(one) - EBIAS

        masked = np.zeros(B, dtype=np.float64)
        for slot, (bt, c, rlo, rhi) in enumerate(op_meta):
            masked[bt * 128 + rlo : bt * 128 + rhi] += msum[rlo:rhi, slot]

        # subtract pad-proto contributions (pad = (PADV, 0, ..., 0))
        d_pad = np.sqrt(x2.astype(np.float64) + PADV * PADV - 2.0 * PADV * xs[:, 0])
        masked -= npad[lab_s] * d_pad

        loss_s = cnt[lab_s] * log_one + masked
        loss = np.empty(B, dtype=np.float64)
        loss[order_b] = loss_s
        return loss.astype(np.float32)

    return _Prep(nc, in_maps, assemble)


def kernel(feature, label, proto_features, proto_labels):
    global LAST_EXEC_NS, LAST_RESULTS
    p = prepare(feature, label, proto_features, proto_labels)
    nc, in_maps = p.nc, p.in_maps

    if SIM:
        from concourse.bass_interp import CoreSim

        results = []
        for k in range(NCORES):
            sim = CoreSim(nc, trace=False)
            for name, arr in in_maps[k].items():
                sim.tensor(name)[:] = arr
            sim.simulate(check_with_hw=False)
            results.append(
                {
                    "onec": sim.tensor("onec").copy(),
                    "msum": sim.tensor("msum").copy(),
                }
            )
        LAST_EXEC_NS = None
    else:
        res = run_bass_kernel_spmd(nc, in_maps, list(range(NCORES)), trace=TRACE)
        results = res.results
        LAST_EXEC_NS = res.exec_time_ns
        global LAST_TRACE
        LAST_TRACE = res.instructions_and_trace
    LAST_RESULTS = results
    return p.assemble(results)
